# revision 22
# baseline (speedup 1.0000x reference)
"""W4A4 quantized linear (AutoQVLALinearW4A4) on 8 Trainium2 NeuronCores.

y = dequant_rowwise_quant(x) @ dequant_w4(qweight)^T + bias

Data-parallel over tokens: each core gets a 512-token slice of x
(row-reversed), the FULL packed weights / scales / bias, and produces the
[512, 4096] output slice for its tokens. No collectives at all — the
per-token amax is local to the core that owns the token, and the host
concatenates the 8 output slices along the token axis.

This replaces the previous column-parallel version whose per-core
steady-state DMA was ~37 MiB/rep (full x replicated to every core, at
~358 GB/s/core that alone is ~100 us). Here steady-state DMA is ~10
MiB/rep/core (own x 4 MiB + q^T transpose 2 MiB + y 4 MiB), so the fp8
matmul pipeline (~123 us roofline) is the only bottleneck.

Device algorithm (per core), exact-integer math on the PE:
  setup (outside the timed repeat loop, like the previous version):
    - unpack FULL int4 weights -> wt_sep [128, C, 2, N] fp8 (128 KiB/part)
    - amax for the 512 own tokens (int16 abs-bit trick + max tree on DVE),
      s = max/7; sq = 1/s; s_flip = J @ s (anti-diagonal partition flip --
      SWI reverses stationary columns, cancelled by the host row-reversal)
  main loop per 128-token tile:
    1. qb = fp16(x * sq + 1536)  (exact round-half-even to int+1536)
       q8 = fp8_e4m3(qb - 1536)  (exact ints in [-8,7])
    2. transpose q8 via DMA-transpose of byte-PAIRS viewed as fp16
    3. for each 512-wide n-block: 16 fp8 DoubleRowSwInterleave matmuls
       (q^T pairs stationary, weight k-planes moving), PSUM [tok, n] exact
    4. epilogue: (psum * a_scale_flip) * wscale + bias on DVE, DMA out
"""

import numpy as np
import concourse.bass as bass
import concourse.mybir as mybir
from concourse import bacc
from concourse.tile import TileContext
from concourse.bass_utils import run_bass_kernel_spmd

F8 = mybir.dt.float8e4
F16 = mybir.dt.float16
F32 = mybir.dt.float32
I8 = mybir.dt.int8
I16 = mybir.dt.int16
AOP = mybir.AluOpType
ACTF = mybir.ActivationFunctionType
SWI = mybir.MatmulPerfMode.DoubleRowSwInterleave

N_CORES = 8


def build(MO=512, K=4096, N=4096, mm_bufs=8, x_bufs=3, qt_bufs=3,
          repeat=1, mm_blk=4, probe=False, **_ignored):
    """Build + compile the per-core program. Returns the Bacc object."""
    assert MO % 128 == 0 and K % 256 == 0 and N % 512 == 0
    TO = MO // 128        # own token tiles (4)
    C = K // 256          # DoubleRow contraction chunks (16)
    KP = K // 2           # packed weight columns (2048)
    NT = N // 128         # weight row tiles to unpack (32)
    NB = N // 512         # 512-wide output blocks per token tile (8)

    nc = bacc.Bacc("TRN2", target_bir_lowering=False, debug=False,
                   num_devices=N_CORES)

    xr_d = nc.dram_tensor("xr", [MO, K], F16, kind="ExternalInput")  # reversed!
    wp_d = nc.dram_tensor("wp", [N, KP], I8, kind="ExternalInput")
    ws_d = nc.dram_tensor("wsc", [1, N], F16, kind="ExternalInput")
    b_d = nc.dram_tensor("bias", [1, N], F16, kind="ExternalInput")
    y_d = nc.dram_tensor("y", [MO, N], F16, kind="ExternalOutput")

    with TileContext(nc) as tc:
        with (
            tc.tile_pool(name="const", bufs=1) as cpool,
            tc.tile_pool(name="wsetup", bufs=1) as wpool,
            tc.tile_pool(name="xwork", bufs=x_bufs) as xpool,
            tc.tile_pool(name="q8p", bufs=2) as q8pool,
            tc.tile_pool(name="qtp", bufs=qt_bufs) as qpool,
            tc.tile_pool(name="small", bufs=2) as spool,
            tc.tile_pool(name="epi", bufs=4) as epool,
            tc.tile_pool(name="psum", bufs=mm_bufs, space="PSUM") as ppool,
        ):
            # ---------------- constants ----------------
            # [1, N] rows are staged in partition 0 of an x-pool buffer so
            # their SBUF space is recycled by the main loop's x tiles.
            wsc_bc = cpool.tile([128, N], F16)
            bias_bc = cpool.tile([128, N], F16)
            row_w = xpool.tile([128, K], F16, tag="x", bufs=x_bufs,
                               name="const_row_w")
            nc.sync.dma_start(row_w[0:1, 0:N], ws_d.ap())
            nc.gpsimd.partition_broadcast(wsc_bc[:, :], row_w[0:1, 0:N])
            row_b = xpool.tile([128, K], F16, tag="x", bufs=x_bufs,
                               name="const_row_b")
            nc.sync.dma_start(row_b[0:1, 0:N], b_d.ap())
            nc.gpsimd.partition_broadcast(bias_bc[:, :], row_b[0:1, 0:N])
            # anti-diagonal J for the partition flip
            jm = cpool.tile([128, 128], F32)
            nc.vector.memset(jm[:, :], 1.0)
            nc.gpsimd.affine_select(jm[:, :], jm[:, :], pattern=[[1, 128]],
                                    base=-127, channel_multiplier=1,
                                    compare_op=AOP.is_equal, fill=0.0)

            # ---------------- weight setup (FULL matrix) ----------------
            # wt_sep[jj, c, i, n] = W[n, 256c + 2jj + i] as fp8 ints
            wt_sep = cpool.tile([128, C, 2, N], F8)
            for nt in range(NT):
                wp_sb = wpool.tile([128, KP], I8, tag="wp")
                nc.sync.dma_start(wp_sb[:, :],
                                  wp_d[nt * 128:(nt + 1) * 128, :])
                w8 = q8pool.tile([128, K], F8, tag="q8", name=f"w8_{nt}")
                w8v = w8[:, :].rearrange("p (j two) -> p j two", two=2)
                # high nibble = floor(b/16) (sign-extended):
                # fp16(b/16 + 1535.53125) - 1536 via exact magic rounding
                hb = wpool.tile([128, KP], F16, tag="hb")
                nc.vector.tensor_scalar(hb[:, :], wp_sb[:, :], 1.0 / 16,
                                        1535.53125, op0=AOP.mult, op1=AOP.add)
                nc.scalar.activation(w8v[:, :, 1], hb[:, :], ACTF.Copy,
                                     bias=-1536.0, scale=1.0)
                # low nibble: ((b & 15) ^ 8) - 8
                lo4 = wpool.tile([128, KP], I8, tag="lo4")
                nc.vector.tensor_scalar(lo4[:, :], wp_sb[:, :], 15, 8,
                                        op0=AOP.bitwise_and,
                                        op1=AOP.bitwise_xor)
                nc.vector.tensor_scalar(w8v[:, :, 0], lo4[:, :], 8.0, None,
                                        op0=AOP.subtract)
                # pair-transpose [128, K]fp8 -> [jj, c, n] pair columns
                wtp = qpool.tile([128, C, 128], F16, tag="qT",
                                 bufs=qt_bufs, name=f"wtp_{nt}")
                nc.scalar.dma_start_transpose(wtp[:, :, :],
                                              w8[:, :].bitcast(F16))
                wtp8 = wtp[:, :, :].bitcast(F8).rearrange(
                    "p c (n two) -> p c two n", two=2)
                nc.scalar.copy(wt_sep[:, :, 0, nt * 128:(nt + 1) * 128],
                               wtp8[:, :, 0, :])
                nc.vector.tensor_copy(wt_sep[:, :, 1, nt * 128:(nt + 1) * 128],
                                      wtp8[:, :, 1, :])

            # ---------------- own-token amax ----------------
            s_own = cpool.tile([128, TO], F32)
            for j in range(TO):
                xt = xpool.tile([128, K], F16, tag="x", bufs=x_bufs,
                                name=f"xam_{j}")
                nc.sync.dma_start(xt[:, :], xr_d[j * 128:(j + 1) * 128, :])
                xa = xt[:, :].bitcast(I16)
                nc.vector.tensor_scalar(xa, xa, 0x7FFF, None,
                                        op0=AOP.bitwise_and)
                w = K // 2
                while w >= 512:
                    nc.vector.tensor_tensor(xt[:, :w].bitcast(I16),
                                            xt[:, :w].bitcast(I16),
                                            xt[:, w:2 * w].bitcast(I16),
                                            op=AOP.max)
                    w //= 2
                mbits = spool.tile([128, 1], I16, tag="mbits")
                nc.vector.tensor_reduce(mbits[:, :],
                                        xt[:, :2 * w].bitcast(I16),
                                        axis=mybir.AxisListType.X,
                                        op=AOP.max)
                nc.vector.tensor_scalar(s_own[:, j:j + 1],
                                        mbits[:, :].bitcast(F16),
                                        1e-6, 1.0 / 7.0,
                                        op0=AOP.max, op1=AOP.mult)

            # reciprocal for quantization + flipped scales for the epilogue
            sq_all = cpool.tile([128, TO], F32)
            nc.vector.reciprocal(sq_all[:, :], s_own[:, :])
            ps_j = ppool.tile([128, 512], F32, tag="mm", bufs=mm_bufs,
                              name="ps_jflip")
            nc.tensor.matmul(ps_j[:, :TO], jm[:, :], s_own[:, :],
                             start=True, stop=True)
            s_flip = cpool.tile([128, TO], F32)
            nc.vector.tensor_copy(s_flip[:, :], ps_j[:, :TO])

            # ---------------- main loop ----------------
            for rep in range(repeat):
              for j in range(TO):
                  xt = xpool.tile([128, K], F16, tag="x", bufs=x_bufs,
                                  name=f"xt_{rep}_{j}")
                  nc.sync.dma_start(xt[:, :], xr_d[j * 128:(j + 1) * 128, :])
                  # qb = fp16(x*sq + 1536): exact RNE integer round
                  nc.vector.tensor_scalar(xt[:, :], xt[:, :],
                                          sq_all[:, j:j + 1], 1536.0,
                                          op0=AOP.mult, op1=AOP.add)
                  # q8 = fp8(qb - 1536), alternating ACT/DVE
                  q8 = q8pool.tile([128, K], F8, tag="q8",
                                   name=f"q8_{rep}_{j}")
                  if j % 2 == 0:
                      nc.scalar.activation(q8[:, :], xt[:, :], ACTF.Copy,
                                           bias=-1536.0, scale=1.0)
                  else:
                      nc.vector.tensor_scalar(q8[:, :], xt[:, :], 1536.0,
                                              None, op0=AOP.subtract)
                  # pair-transpose: qT[jj, c, f] = (q[f, 256c+2jj], +1)
                  qT = qpool.tile([128, C, 128], F16, tag="qT",
                                  bufs=qt_bufs, name=f"qT_{rep}_{j}")
                  nc.scalar.dma_start_transpose(qT[:, :, :],
                                                q8[:, :].bitcast(F16))
                  qT8 = qT[:, :, :].bitcast(F8)  # [128, C, 256]

                  # bank-blocked matmul order: each stationary qT8[:, c, :]
                  # is reused across mm_blk consecutive matmuls (amortizes
                  # LDWEIGHTS) while only mm_blk PSUM banks stay live, so
                  # epilogues overlap the next block's matmuls.
                  for b0 in range(0, NB, mm_blk):
                      ps_b = [ppool.tile([128, 512], F32, tag="mm",
                                         bufs=mm_bufs,
                                         name=f"ps_{rep}_{j}_{nb}")
                              for nb in range(b0, b0 + mm_blk)]
                      C_mm = C // 2 if probe else C
                      for c in range(C_mm):
                          for bi in range(mm_blk):
                              nb = b0 + bi
                              nc.tensor.matmul(
                                  ps_b[bi][:, :], qT8[:, c, :],
                                  wt_sep[:, c, :, nb * 512:(nb + 1) * 512],
                                  start=(c == 0), stop=(c == C_mm - 1),
                                  perf_mode=SWI)
                      for bi in range(mm_blk):
                          nb = b0 + bi
                          # epilogue: y = (ps * a_scale) * wscale + bias
                          # ACT does the PSUM read + per-token scale; DVE
                          # does the two cheap fp16 SBUF ops.
                          t0 = epool.tile([128, 512], F16, tag="t0",
                                          name=f"t0_{rep}_{j}_{nb}")
                          nc.scalar.activation(t0[:, :], ps_b[bi][:, :],
                                               ACTF.Copy,
                                               scale=s_flip[:, j:j + 1])
                          t1 = epool.tile([128, 512], F16, tag="t1",
                                          name=f"t1_{rep}_{j}_{nb}")
                          nc.vector.tensor_tensor(
                              t1[:, :], t0[:, :],
                              wsc_bc[:, nb * 512:(nb + 1) * 512], op=AOP.mult)
                          nc.vector.tensor_tensor(
                              t1[:, :], t1[:, :],
                              bias_bc[:, nb * 512:(nb + 1) * 512], op=AOP.add)
                          nc.sync.dma_start(
                              y_d[MO - 128 * (j + 1):MO - 128 * j,
                                  nb * 512:(nb + 1) * 512], t1[:, :])

    nc.compile()
    return nc


_CACHE = {}


def _get_nc():
    if "nc" not in _CACHE:
        _CACHE["nc"] = build()
    return _CACHE["nc"]


def _in_maps(x, qweight_packed, w_scales, bias):
    M, K, N = 4096, 4096, 4096
    MO = M // N_CORES
    x2 = np.asarray(x).reshape(M, K)
    wp = np.ascontiguousarray(np.asarray(qweight_packed))
    wsc = np.ascontiguousarray(np.asarray(w_scales).reshape(1, N))
    b = np.ascontiguousarray(np.asarray(bias).reshape(1, N))
    in_maps = []
    for c in range(N_CORES):
        in_maps.append({
            "xr": np.ascontiguousarray(x2[c * MO:(c + 1) * MO][::-1]),
            "wp": wp,
            "wsc": wsc,
            "bias": b,
        })
    return in_maps


def run_traced(x, qweight_packed, w_scales, bias, tmpdir=None):
    nc = _get_nc()
    in_maps = _in_maps(x, qweight_packed, w_scales, bias)
    return run_bass_kernel_spmd(nc, in_maps, core_ids=list(range(N_CORES)),
                                trace=True, tmpdir=tmpdir)


def kernel(x, qweight_packed, w_scales, bias):
    M, N = 4096, 4096
    MO = M // N_CORES
    nc = _get_nc()
    in_maps = _in_maps(x, qweight_packed, w_scales, bias)
    res = run_bass_kernel_spmd(nc, in_maps, core_ids=list(range(N_CORES)))
    y = np.concatenate([res.results[c]["y"] for c in range(N_CORES)], axis=0)
    return y.reshape(2, 2048, N)
